# revision 15
# baseline (speedup 1.0000x reference)
"""Trainium2 Bass kernel for nn_Block_27187142983954 (dense transformer block,
per-position head-mixing attention). Data-parallel over batch: 8 cores, one
batch element each. Self-contained: hardcodes all shapes.

Per-core plan (S=4096 positions, E=1024, H=16 heads, D=64), software-pipelined
across position tiles; engine queues are kept stall-free:
  - VectorE: ONLY the big attention bilinears (bf16 2x-mode broadcast mults +
    first two halving-tree levels) plus tiny den/recip/phat ops, per head-half
    so softmax of half 0 overlaps the QK of half 1.
  - GpSimd: tree tails (levels 3+ of both reductions), LN scalar arithmetic,
    second residual add.
  - TensorE: qkv/proj/ff matmuls (x feature-major stationary), PE transposes
    packed 8-to-a-PSUM-bank, proj's +x residual accumulated in PSUM via
    identity-matrix matmuls. qkv emitted first each iteration as the PE
    gap-filler; tile t-1's proj/ff follow.
  - ScalarE: all PSUM evacuations (merged), per-half softmax exp placed
    BEFORE the v-group evacuations so VectorE never waits on it, LN stats
    via activation accum_out, gelu (2 act-table loads/tile).
  - softmax probs are pre-normalized (phat = exp * 1/den in bf16) so the AV
    tree output is the attention output directly.
  - LN1 reads straight from PSUM; ln_g/ln_b of LN1 folded into ff weights on
    the host; final LN affine skipped when ln_g==1, ln_b==0 (true here).
"""

import sys

sys.path.insert(0, "/opt/trn_rl_repo")

import numpy as np
import ml_dtypes

E, H, DQ, DV = 1024, 16, 64, 64
B, S = 8, 4096
EPS = 1e-5
NT = S // 128  # 32 position tiles per core
BF = ml_dtypes.bfloat16

_CACHE = {}


def _patch_tail_drain():
    """walrus in this container rejects >1 sem wait on a CTRL (Drain)
    instruction; spread the TileContext tail-drain waits over wait-nops."""
    import concourse.tile as tile
    import bass_rust
    from concourse.vector_clock import ScopedClock

    if getattr(tile.TileContext, "_drain_patched", False):
        return

    def _drain_and_barrier(self, tick_clock, wait_clock):
        nc = self.nc
        drain_inst = nc.sync.drain()
        wait_clock.add_sem_waits(
            drain_inst.ins, ScopedClock({None: tick_clock.global_clock})
        )
        si = drain_inst.ins.sync_info
        waits = list(si.on_wait) if si is not None else []
        if len(waits) > 1:
            drain_inst.ins.sync_info = bass_rust.SyncInfo(on_wait=[], on_update=[])
            for w in waits:
                nop = nc.sync.nop()
                nop.ins.sync_info = bass_rust.SyncInfo(on_wait=[w], on_update=[])
        nc.all_engine_barrier()
        assert self.sems is not None
        popped = nc._tile_sem_poison_stack.pop()
        assert popped is self._sem_poison
        nc.clear_and_free_semaphores(list(self.sems.allocated().values()))
        nc.all_engine_barrier()

    tile.TileContext._drain_and_barrier = _drain_and_barrier
    tile.TileContext._drain_patched = True


def _split_excess_waits(nc, max_on_op=1, max_on_nop=1):
    """walrus in this container rejects >2 sem waits on compute instruction
    structs and >1 on DMA/CTRL structs. Hoist excess waits onto preceding
    same-engine NOPs."""
    import concourse.mybir as mybir
    import bass_rust

    narrow = {"DMACopy", "Drain", "NoOp", "Memset", "TriggeredCopy"}
    cnt = 0
    for bb in nc.m.functions[0].blocks:
        il = bb.instructions
        out = []
        for inst in il:
            cap = 1 if inst.opcode in narrow else max_on_op
            si = inst.sync_info
            waits = list(si.on_wait) if si is not None and si.on_wait else []
            if len(waits) > cap:
                n_extra = len(waits) - cap
                extra, keep = waits[:n_extra], waits[n_extra:]
                for i0 in range(0, len(extra), max_on_nop):
                    chunk = extra[i0 : i0 + max_on_nop]
                    nop = mybir.InstNoOp(name=f"waitnop-{cnt}", ins=[], outs=[])
                    cnt += 1
                    nop.engine = inst.engine
                    nop.sync_info = bass_rust.SyncInfo(on_wait=chunk, on_update=[])
                    out.append(nop)
                inst.sync_info = bass_rust.SyncInfo(
                    on_wait=keep,
                    on_update=list(si.on_update) if si.on_update else [],
                )
            out.append(inst)
        il[:] = out


def _build_program(apply_gb: bool):
    import concourse.bass as bass
    import concourse.tile as tile
    import concourse.mybir as mybir
    from concourse.masks import make_identity

    _patch_tail_drain()

    f32 = mybir.dt.float32
    bf16 = mybir.dt.bfloat16
    ALU = mybir.AluOpType
    ACT = mybir.ActivationFunctionType

    nc = bass.Bass("TRN2", target_bir_lowering=False, debug=False, num_devices=1)

    x_bf_d = nc.dram_tensor("x_bf", [S, E], bf16, kind="ExternalInput").ap()
    xT = nc.dram_tensor("xT", [E, S], bf16, kind="ExternalInput").ap()
    wqkvT_d = nc.dram_tensor("wqkvT", [E, 3 * E], bf16, kind="ExternalInput").ap()
    projT_d = nc.dram_tensor("projT", [E, E], bf16, kind="ExternalInput").ap()
    ffw2T_d = nc.dram_tensor("ffw2T", [E, E], bf16, kind="ExternalInput").ap()
    bqkv_d = nc.dram_tensor("bqkv", [1, 3 * E], bf16, kind="ExternalInput").ap()
    bproj_d = nc.dram_tensor("bproj", [1, E], bf16, kind="ExternalInput").ap()
    bff2_d = nc.dram_tensor("bff2", [1, E], bf16, kind="ExternalInput").ap()
    if apply_gb:
        g_rep_d = nc.dram_tensor("g_rep", [128, E], f32, kind="ExternalInput").ap()
        b_rep_d = nc.dram_tensor("b_rep", [128, E], f32, kind="ExternalInput").ap()
    out_d = nc.dram_tensor("out", [S, E], f32, kind="ExternalOutput").ap()

    xT_r = xT.rearrange("(t p) s -> p t s", p=128)  # [128, 8, S]
    wqkv_r = wqkvT_d.rearrange("(t p) o -> p t o", p=128)
    proj_r = projT_d.rearrange("(t p) o -> p t o", p=128)
    ffw2_r = ffw2T_d.rearrange("(t p) o -> p t o", p=128)

    inv_n = 1.0 / float(E)

    with tile.TileContext(nc) as tc:
        import contextlib

        ctx = contextlib.ExitStack()
        with ctx:
            fixed = ctx.enter_context(tc.tile_pool(name="fixed", bufs=1))
            work = ctx.enter_context(tc.tile_pool(name="work", bufs=2))
            work1 = ctx.enter_context(tc.tile_pool(name="work1", bufs=1))
            workq = ctx.enter_context(tc.tile_pool(name="workq", bufs=2))
            stats = ctx.enter_context(tc.tile_pool(name="stats", bufs=8))
            psq = ctx.enter_context(tc.tile_pool(name="psq", bufs=2, space="PSUM"))
            psb = ctx.enter_context(tc.tile_pool(name="psb", bufs=2, space="PSUM"))
            pst = ctx.enter_context(tc.tile_pool(name="pst", bufs=2, space="PSUM"))

            # ---- fixed tensors ----
            wqkv_sb = fixed.tile([128, 8, 3 * E], bf16)
            for t in range(8):
                nc.sync.dma_start(out=wqkv_sb[:, t, :], in_=wqkv_r[:, t, :])
            proj_sb = fixed.tile([128, 8, E], bf16)
            ffw2_sb = fixed.tile([128, 8, E], bf16)
            for t in range(8):
                nc.sync.dma_start(out=proj_sb[:, t, :], in_=proj_r[:, t, :])
                nc.sync.dma_start(out=ffw2_sb[:, t, :], in_=ffw2_r[:, t, :])
            bqkv_sb = fixed.tile([1, 3 * E], bf16)
            nc.sync.dma_start(out=bqkv_sb, in_=bqkv_d)
            bproj_sb = fixed.tile([1, E], bf16)
            nc.sync.dma_start(out=bproj_sb, in_=bproj_d)
            bff2_sb = fixed.tile([1, E], bf16)
            nc.sync.dma_start(out=bff2_sb, in_=bff2_d)
            if apply_gb:
                g_rep = fixed.tile([128, E], f32)
                nc.sync.dma_start(out=g_rep, in_=g_rep_d)
                b_rep = fixed.tile([128, E], f32)
                nc.sync.dma_start(out=b_rep, in_=b_rep_d)
            ones_row = fixed.tile([1, 128], bf16)
            nc.vector.memset(ones_row, 1.0)
            ident = fixed.tile([128, 128], bf16)
            make_identity(nc, ident)
            eps_sb = fixed.tile([128, 1], f32)
            nc.vector.memset(eps_sb, EPS)

            def layer_norm_rs(z, rs_out, mrs_out, scratch_bf):
                """rsigma and -mu*rsigma of z [128, E]; stats on ScalarE,
                scalar TT arithmetic on GpSimd."""
                s1 = stats.tile([128, 1], f32, tag="s1")
                s2 = stats.tile([128, 1], f32, tag="s2")
                nc.scalar.activation(scratch_bf, z, ACT.Identity, accum_out=s1)
                nc.scalar.activation(scratch_bf, z, ACT.Square, accum_out=s2)
                mun = stats.tile([128, 1], f32, tag="mun")  # -mu
                nc.scalar.activation(mun, s1, ACT.Identity, scale=-inv_n)
                s2n = stats.tile([128, 1], f32, tag="s2n")  # s2/N
                nc.scalar.activation(s2n, s2, ACT.Identity, scale=inv_n)
                a = stats.tile([128, 1], f32, tag="a")  # mu^2
                nc.gpsimd.tensor_tensor(a, mun, mun, ALU.mult)
                var = stats.tile([128, 1], f32, tag="var")
                nc.gpsimd.tensor_tensor(var, s2n, a, ALU.subtract)
                lnv = stats.tile([128, 1], f32, tag="lnv")
                nc.scalar.activation(lnv, var, ACT.Ln, bias=eps_sb)
                nc.scalar.activation(rs_out, lnv, ACT.Exp, scale=-0.5)
                nc.gpsimd.tensor_tensor(mrs_out, mun, rs_out, ALU.mult)

            prev = None
            for t in range(NT + 1):
                cur = None
                if t < NT:
                    s0 = t * 128
                    xf = work.tile([128, 8, 128], bf16, tag="xf")
                    nc.sync.dma_start(out=xf, in_=xT_r[:, :, s0 : s0 + 128])
                    xb = work.tile([128, E], bf16, tag="xb")
                    nc.sync.dma_start(out=xb, in_=x_bf_d[s0 : s0 + 128, :])
                    cur = {"s0": s0, "xf": xf, "xb": xb}

                    # ---- qkv q,k groups first (PE) + evacs (Scalar) ----
                    qkv_sb = workq.tile([128, 3 * E], bf16, tag="qkv")

                    def qkv_group(j):
                        ps = psq.tile([128, 512], f32, tag="psq")
                        for e in range(8):
                            nc.tensor.matmul(
                                ps,
                                xf[:, e, :],
                                wqkv_sb[:, e, j * 512 : (j + 1) * 512],
                                start=(e == 0),
                                stop=False,
                            )
                        nc.tensor.matmul(
                            ps,
                            ones_row,
                            bqkv_sb[:, j * 512 : (j + 1) * 512],
                            start=False,
                            stop=True,
                        )
                        nc.scalar.copy(qkv_sb[:, j * 512 : (j + 1) * 512], ps)

                    for j in (0, 2, 3, 1):
                        qkv_group(j)

                    q3 = qkv_sb[:, 0:E].rearrange("p (h d) -> p h d", h=H)
                    k3 = qkv_sb[:, E : 2 * E].rearrange("p (g d) -> p g d", g=H)
                    v3 = qkv_sb[:, 2 * E : 3 * E].rearrange(
                        "p (d g) -> p d g", d=DV
                    )

                    # ---- QK bilinear per half (all on VectorE) ----
                    prod = work1.tile([128, 8, 16, 64], bf16, tag="prod")
                    scr = work1.tile([128, 8192], bf16, tag="scr")
                    scores = work.tile([128, H, H], f32, tag="scores")
                    p_sb = work.tile([128, H, H], bf16, tag="p_sb")
                    t1 = scr[:, 0:4096].rearrange("p (a g d) -> p a g d", a=8, g=16)
                    t2 = scr[:, 4096:6144].rearrange(
                        "p (a g d) -> p a g d", a=8, g=16
                    )
                    t3 = scr[:, 6144:7168].rearrange(
                        "p (a g d) -> p a g d", a=8, g=16
                    )

                    def qk_half(half):
                        h0 = half * 8
                        qb = (
                            q3[:, h0 : h0 + 8, :]
                            .unsqueeze(2)
                            .broadcast_to([128, 8, 16, 64])
                        )
                        kb = k3.unsqueeze(1).broadcast_to([128, 8, 16, 64])
                        nc.vector.tensor_tensor(prod, kb, qb, ALU.mult)
                        nc.vector.tensor_tensor(
                            t1, prod[:, :, :, 0:32], prod[:, :, :, 32:64], ALU.add
                        )
                        nc.vector.tensor_tensor(
                            t2, t1[:, :, :, 0:16], t1[:, :, :, 16:32], ALU.add
                        )
                        nc.vector.tensor_tensor(
                            t3, t2[:, :, :, 0:8], t2[:, :, :, 8:16], ALU.add
                        )
                        nc.vector.tensor_reduce(
                            scores[:, h0 : h0 + 8, :],
                            t3,
                            axis=mybir.AxisListType.X,
                            op=ALU.add,
                        )
                        # softmax exp for this half (Scalar)
                        nc.scalar.activation(
                            p_sb[:, h0 : h0 + 8, :],
                            scores[:, h0 : h0 + 8, :],
                            ACT.Exp,
                        )

                    qk_half(0)
                    qk_half(1)

                    # v groups (PE) + evacs after the exps in Scalar's queue
                    for j in (4, 5):
                        qkv_group(j)

                    # ---- softmax normalize + AV per half ----
                    attn_bf = work.tile([128, E], bf16, tag="attn_bf")
                    a3v = attn_bf.rearrange("p (h d) -> p h d", h=H)
                    phat = work1.tile([128, 2, 8, 16], bf16, tag="phat")
                    u1 = scr[:, 0:4096].rearrange("p (a d g) -> p a d g", a=8, d=64)
                    u2 = scr[:, 4096:6144].rearrange(
                        "p (a d g) -> p a d g", a=8, d=64
                    )
                    prod_flat = prod.rearrange("p a g d -> p (a g d)")
                    pa = prod_flat.rearrange("p (a d g) -> p a d g", a=8, d=DV)

                    def av_half(half):
                        h0 = half * 8
                        den = stats.tile([128, 8], f32, tag=f"den{half}")
                        nc.vector.tensor_reduce(
                            den,
                            p_sb[:, h0 : h0 + 8, :],
                            axis=mybir.AxisListType.X,
                            op=ALU.add,
                        )
                        rden = stats.tile([128, 8], bf16, tag=f"rden{half}")
                        with nc.allow_low_precision(reason="1/den in bf16"):
                            nc.vector.reciprocal(rden, den)
                        rb = rden.unsqueeze(2).broadcast_to([128, 8, 16])
                        nc.vector.tensor_tensor(
                            phat[:, half], p_sb[:, h0 : h0 + 8, :], rb, ALU.mult
                        )
                        pb = (
                            phat[:, half]
                            .unsqueeze(2)
                            .broadcast_to([128, 8, 64, 16])
                        )
                        vb = v3.unsqueeze(1).broadcast_to([128, 8, 64, 16])
                        nc.vector.tensor_tensor(pa, vb, pb, ALU.mult)
                        nc.vector.tensor_tensor(
                            u1, pa[:, :, :, 0:8], pa[:, :, :, 8:16], ALU.add
                        )
                        nc.vector.tensor_tensor(
                            u2, u1[:, :, :, 0:4], u1[:, :, :, 4:8], ALU.add
                        )
                        u3 = scr[:, 6144:7168].rearrange(
                            "p (a d g) -> p a d g", a=8, d=64
                        )
                        nc.vector.tensor_tensor(
                            u3, u2[:, :, :, 0:2], u2[:, :, :, 2:4], ALU.add
                        )
                        with nc.allow_low_precision(reason="2-term sum in bf16"):
                            nc.vector.tensor_reduce(
                                a3v[:, h0 : h0 + 8, :],
                                u3,
                                axis=mybir.AxisListType.X,
                                op=ALU.add,
                            )

                    av_half(0)
                    av_half(1)
                    cur["attn_bf"] = attn_bf

                if prev is not None:
                    p = prev
                    pxf, pxb = p["xf"], p["xb"]
                    # ---- attn transposes (packed psum bank) + evac ----
                    ptk = pst.tile([128, 8, 128], bf16, tag="pt")
                    for e in range(8):
                        nc.tensor.transpose(
                            ptk[:, e, :],
                            p["attn_bf"][:, e * 128 : (e + 1) * 128],
                            ident,
                        )
                    attn_fm = work.tile([128, 8, 128], bf16, tag="attn_fm")
                    nc.scalar.copy(
                        attn_fm.rearrange("p a s -> p (a s)"),
                        ptk.rearrange("p a s -> p (a s)"),
                    )
                    # ---- proj + bias + residual in PSUM ----
                    ps2 = psb.tile([128, 1024], f32, tag="psb")
                    for j in range(2):
                        for e in range(8):
                            nc.tensor.matmul(
                                ps2[:, j * 512 : (j + 1) * 512],
                                attn_fm[:, e, :],
                                proj_sb[:, e, j * 512 : (j + 1) * 512],
                                start=(e == 0),
                                stop=False,
                            )
                        for c in range(4):
                            ec = 4 * j + c
                            nc.tensor.matmul(
                                ps2[:, ec * 128 : (ec + 1) * 128],
                                pxf[:, ec, :],
                                ident,
                                start=False,
                                stop=False,
                                skip_group_check=True,
                            )
                        nc.tensor.matmul(
                            ps2[:, j * 512 : (j + 1) * 512],
                            ones_row,
                            bproj_sb[:, j * 512 : (j + 1) * 512],
                            start=False,
                            stop=True,
                        )
                    # ---- LN1 from PSUM ----
                    lnscr = work1.tile([128, E], bf16, tag="lnscr")
                    rs1 = stats.tile([128, 1], f32, tag="rs1")
                    mrs1 = stats.tile([128, 1], f32, tag="mrs1")
                    layer_norm_rs(ps2, rs1, mrs1, lnscr)
                    ln1_bf = work.tile([128, E], bf16, tag="ln1_bf")
                    nc.scalar.activation(
                        ln1_bf, ps2, ACT.Identity, bias=mrs1, scale=rs1
                    )
                    ptk2 = pst.tile([128, 8, 128], bf16, tag="pt")
                    for e in range(8):
                        nc.tensor.transpose(
                            ptk2[:, e, :], ln1_bf[:, e * 128 : (e + 1) * 128], ident
                        )
                    ln1_fm = work.tile([128, 8, 128], bf16, tag="ln1_fm")
                    nc.scalar.copy(
                        ln1_fm.rearrange("p a s -> p (a s)"),
                        ptk2.rearrange("p a s -> p (a s)"),
                    )
                    # ---- ff + gelu ----
                    ps3 = psb.tile([128, 1024], f32, tag="psb")
                    for j in range(2):
                        for e in range(8):
                            nc.tensor.matmul(
                                ps3[:, j * 512 : (j + 1) * 512],
                                ln1_fm[:, e, :],
                                ffw2_sb[:, e, j * 512 : (j + 1) * 512],
                                start=(e == 0),
                                stop=False,
                            )
                        nc.tensor.matmul(
                            ps3[:, j * 512 : (j + 1) * 512],
                            ones_row,
                            bff2_sb[:, j * 512 : (j + 1) * 512],
                            start=False,
                            stop=True,
                        )
                    gl = work.tile([128, E], bf16, tag="gl")
                    nc.scalar.activation(gl, ps3, ACT.Gelu)
                    # ---- second residual (VectorE; queued after attention) ----
                    z2 = work.tile([128, E], bf16, tag="z2")
                    nc.vector.tensor_tensor(z2, gl, pxb, ALU.add)
                    rs2 = stats.tile([128, 1], f32, tag="rs2")
                    mrs2 = stats.tile([128, 1], f32, tag="mrs2")
                    layer_norm_rs(z2, rs2, mrs2, lnscr)
                    out_t = work.tile([128, E], f32, tag="out_t")
                    if apply_gb:
                        zn = work1.tile([128, E], f32, tag="zn")
                        nc.scalar.activation(
                            zn, z2, ACT.Identity, bias=mrs2, scale=rs2
                        )
                        zn2 = work1.tile([128, E], f32, tag="zn2")
                        nc.gpsimd.tensor_tensor(zn2, zn, g_rep, ALU.mult)
                        nc.gpsimd.tensor_tensor(out_t, zn2, b_rep, ALU.add)
                    else:
                        nc.scalar.activation(
                            out_t, z2, ACT.Identity, bias=mrs2, scale=rs2
                        )
                    nc.sync.dma_start(
                        out=out_d[p["s0"] : p["s0"] + 128, :], in_=out_t
                    )

                prev = cur

    _split_excess_waits(nc)
    return nc


def _host_prep(inputs):
    x = np.asarray(inputs["x"], np.float32)
    qk_w = np.asarray(inputs["qk_w"], np.float32)
    qk_b = np.asarray(inputs["qk_b"], np.float32)
    v_w = np.asarray(inputs["v_w"], np.float32)
    v_b = np.asarray(inputs["v_b"], np.float32)
    proj_w = np.asarray(inputs["proj_w"], np.float32)
    proj_b = np.asarray(inputs["proj_b"], np.float32)
    ff_w = np.asarray(inputs["ff_w"], np.float32)
    ff_b = np.asarray(inputs["ff_b"], np.float32)
    ln_g = np.asarray(inputs["ln_g"], np.float32)
    ln_b = np.asarray(inputs["ln_b"], np.float32)

    apply_gb = not (np.all(ln_g == 1.0) and np.all(ln_b == 0.0))

    scale = 1.0 / np.sqrt(DQ).astype(np.float32)
    Wq = qk_w[:E] * scale
    bq = qk_b[:E] * scale
    Wk = qk_w[E:]
    bk = qk_b[E:]
    g_idx, d_idx = np.meshgrid(np.arange(H), np.arange(DV), indexing="ij")
    perm = np.empty(E, np.int64)
    perm[(d_idx * H + g_idx).ravel()] = (g_idx * DV + d_idx).ravel()
    Wv2 = v_w[perm]
    bv2 = v_b[perm]

    wqkvT = np.ascontiguousarray(
        np.concatenate([Wq, Wk, Wv2], 0).T.astype(BF)
    )  # [E, 3E]
    bqkv = np.concatenate([bq, bk, bv2])[None, :].astype(BF)  # [1, 3E]
    projT = np.ascontiguousarray(proj_w.T.astype(BF))  # [E, E]
    bproj = proj_b[None, :].astype(BF)
    ffw2T = np.ascontiguousarray((ff_w * ln_g[None, :]).T.astype(BF))
    bff2 = (ff_b + ff_w @ ln_b)[None, :].astype(BF)

    shared = {
        "wqkvT": wqkvT,
        "bqkv": bqkv,
        "projT": projT,
        "bproj": bproj,
        "ffw2T": ffw2T,
        "bff2": bff2,
    }
    if apply_gb:
        shared["g_rep"] = np.ascontiguousarray(
            np.broadcast_to(ln_g[None, :], (128, E)), np.float32
        )
        shared["b_rep"] = np.ascontiguousarray(
            np.broadcast_to(ln_b[None, :], (128, E)), np.float32
        )
    in_maps = []
    for b in range(B):
        xb = np.ascontiguousarray(x[b])  # [S, E] f32
        m = {
            "x_bf": xb.astype(BF),
            "xT": np.ascontiguousarray(xb.T.astype(BF)),
        }
        m.update(shared)
        in_maps.append(m)
    return in_maps, apply_gb


def kernel(**inputs) -> np.ndarray:
    from concourse.bass_utils import run_bass_kernel_spmd

    in_maps, apply_gb = _host_prep(inputs)
    key = ("nc", apply_gb)
    if key not in _CACHE:
        _CACHE[key] = _build_program(apply_gb)
        _CACHE["nc"] = _CACHE[key]  # convenience alias for external harnesses
    nc = _CACHE[key]

    res = run_bass_kernel_spmd(nc, in_maps, core_ids=list(range(B)))
    out = np.stack([res.results[b]["out"] for b in range(B)], 0)
    return out.astype(np.float32)


if __name__ == "__main__":
    rng = np.random.default_rng(0)
    ins = {
        "x": rng.standard_normal((B, S, E), np.float32),
        "qk_w": rng.standard_normal((2 * E, E), np.float32) * 0.03,
        "qk_b": rng.standard_normal((2 * E,), np.float32) * 0.03,
        "v_w": rng.standard_normal((E, E), np.float32) * 0.03,
        "v_b": rng.standard_normal((E,), np.float32) * 0.03,
        "proj_w": rng.standard_normal((E, E), np.float32) * 0.03,
        "proj_b": rng.standard_normal((E,), np.float32) * 0.03,
        "ff_w": rng.standard_normal((E, E), np.float32) * 0.03,
        "ff_b": rng.standard_normal((E,), np.float32) * 0.03,
        "ln_g": np.ones((E,), np.float32),
        "ln_b": np.zeros((E,), np.float32),
    }
    o = kernel(**ins)
    print("ran", o.shape, o.dtype)


# revision 18
# speedup vs baseline: 1.1459x; 1.1459x over previous
"""Trainium2 Bass kernel for nn_Block_27187142983954 (dense transformer block,
per-position head-mixing attention). Data-parallel over batch: 8 cores, one
batch element each. Self-contained: hardcodes all shapes.

Per-core plan (S=4096 positions, E=1024, H=16 heads, D=64), software-pipelined
across position tiles; engine queues are kept stall-free:
  - VectorE: ONLY the big attention bilinears (bf16 2x-mode broadcast mults +
    first two halving-tree levels) plus tiny den/recip/phat ops, per head-half
    so softmax of half 0 overlaps the QK of half 1.
  - GpSimd: tree tails (levels 3+ of both reductions), LN scalar arithmetic,
    second residual add.
  - TensorE: qkv/proj/ff matmuls (x feature-major stationary), PE transposes
    packed 8-to-a-PSUM-bank, proj's +x residual accumulated in PSUM via
    identity-matrix matmuls. qkv emitted first each iteration as the PE
    gap-filler; tile t-1's proj/ff follow.
  - ScalarE: all PSUM evacuations (merged), per-half softmax exp placed
    BEFORE the v-group evacuations so VectorE never waits on it, LN stats
    via activation accum_out, gelu (2 act-table loads/tile).
  - softmax probs are pre-normalized (phat = exp * 1/den in bf16) so the AV
    tree output is the attention output directly.
  - LN1 reads straight from PSUM; ln_g/ln_b of LN1 folded into ff weights on
    the host; final LN affine skipped when ln_g==1, ln_b==0 (true here).
"""

import sys

sys.path.insert(0, "/opt/trn_rl_repo")

import numpy as np
import ml_dtypes

E, H, DQ, DV = 1024, 16, 64, 64
B, S = 8, 4096
EPS = 1e-5
NT = S // 128  # 32 position tiles per core
BF = ml_dtypes.bfloat16

_CACHE = {}


def _patch_tail_drain():
    """walrus in this container rejects >1 sem wait on a CTRL (Drain)
    instruction; spread the TileContext tail-drain waits over wait-nops."""
    import concourse.tile as tile
    import bass_rust
    from concourse.vector_clock import ScopedClock

    if getattr(tile.TileContext, "_drain_patched", False):
        return

    def _drain_and_barrier(self, tick_clock, wait_clock):
        nc = self.nc
        drain_inst = nc.sync.drain()
        wait_clock.add_sem_waits(
            drain_inst.ins, ScopedClock({None: tick_clock.global_clock})
        )
        si = drain_inst.ins.sync_info
        waits = list(si.on_wait) if si is not None else []
        if len(waits) > 1:
            drain_inst.ins.sync_info = bass_rust.SyncInfo(on_wait=[], on_update=[])
            for w in waits:
                nop = nc.sync.nop()
                nop.ins.sync_info = bass_rust.SyncInfo(on_wait=[w], on_update=[])
        nc.all_engine_barrier()
        assert self.sems is not None
        popped = nc._tile_sem_poison_stack.pop()
        assert popped is self._sem_poison
        nc.clear_and_free_semaphores(list(self.sems.allocated().values()))
        nc.all_engine_barrier()

    tile.TileContext._drain_and_barrier = _drain_and_barrier
    tile.TileContext._drain_patched = True


def _split_excess_waits(nc, max_on_op=1, max_on_nop=1):
    """walrus in this container rejects >2 sem waits on compute instruction
    structs and >1 on DMA/CTRL structs. Hoist excess waits onto preceding
    same-engine NOPs."""
    import concourse.mybir as mybir
    import bass_rust

    narrow = {"DMACopy", "Drain", "NoOp", "Memset", "TriggeredCopy"}
    cnt = 0
    for bb in nc.m.functions[0].blocks:
        il = bb.instructions
        out = []
        for inst in il:
            cap = 1 if inst.opcode in narrow else max_on_op
            si = inst.sync_info
            waits = list(si.on_wait) if si is not None and si.on_wait else []
            if len(waits) > cap:
                n_extra = len(waits) - cap
                extra, keep = waits[:n_extra], waits[n_extra:]
                for i0 in range(0, len(extra), max_on_nop):
                    chunk = extra[i0 : i0 + max_on_nop]
                    nop = mybir.InstNoOp(name=f"waitnop-{cnt}", ins=[], outs=[])
                    cnt += 1
                    nop.engine = inst.engine
                    nop.sync_info = bass_rust.SyncInfo(on_wait=chunk, on_update=[])
                    out.append(nop)
                inst.sync_info = bass_rust.SyncInfo(
                    on_wait=keep,
                    on_update=list(si.on_update) if si.on_update else [],
                )
            out.append(inst)
        il[:] = out


def _build_program(apply_gb: bool):
    import concourse.bass as bass
    import concourse.tile as tile
    import concourse.mybir as mybir
    from concourse.masks import make_identity

    _patch_tail_drain()

    f32 = mybir.dt.float32
    bf16 = mybir.dt.bfloat16
    ALU = mybir.AluOpType
    ACT = mybir.ActivationFunctionType

    nc = bass.Bass("TRN2", target_bir_lowering=False, debug=False, num_devices=1)

    f8 = mybir.dt.float8e4
    x_bf_d = nc.dram_tensor("x_bf", [S, E], bf16, kind="ExternalInput").ap()
    xT = nc.dram_tensor("xT", [E, S], bf16, kind="ExternalInput").ap()
    x8T = nc.dram_tensor("x8T", [E, S], f8, kind="ExternalInput").ap()
    wqkvT_d = nc.dram_tensor("wqkvT", [E, 3 * E], f8, kind="ExternalInput").ap()
    projT_d = nc.dram_tensor("projT", [E, E], f8, kind="ExternalInput").ap()
    ffw2T_d = nc.dram_tensor("ffw2T", [E, E], bf16, kind="ExternalInput").ap()
    bqkv_d = nc.dram_tensor("bqkv", [1, 3 * E], bf16, kind="ExternalInput").ap()
    bproj_d = nc.dram_tensor("bproj", [1, E], bf16, kind="ExternalInput").ap()
    bff2_d = nc.dram_tensor("bff2", [1, E], bf16, kind="ExternalInput").ap()
    if apply_gb:
        g_rep_d = nc.dram_tensor("g_rep", [128, E], f32, kind="ExternalInput").ap()
        b_rep_d = nc.dram_tensor("b_rep", [128, E], f32, kind="ExternalInput").ap()
    out_d = nc.dram_tensor("out", [S, E], f32, kind="ExternalOutput").ap()

    xT_r = xT.rearrange("(t p) s -> p t s", p=128)  # [128, 8, S]
    x8T_r = x8T.rearrange("(t p) s -> p t s", p=128)
    wqkv_r = wqkvT_d.rearrange("(t p) o -> p t o", p=128)
    proj_r = projT_d.rearrange("(t p) o -> p t o", p=128)
    ffw2_r = ffw2T_d.rearrange("(t p) o -> p t o", p=128)

    inv_n = 1.0 / float(E)

    with tile.TileContext(nc) as tc:
        import contextlib

        ctx = contextlib.ExitStack()
        with ctx:
            fixed = ctx.enter_context(tc.tile_pool(name="fixed", bufs=1))
            work = ctx.enter_context(tc.tile_pool(name="work", bufs=2))
            work1 = ctx.enter_context(tc.tile_pool(name="work1", bufs=1))
            workq = ctx.enter_context(tc.tile_pool(name="workq", bufs=2))
            stats = ctx.enter_context(tc.tile_pool(name="stats", bufs=8))
            psq = ctx.enter_context(tc.tile_pool(name="psq", bufs=2, space="PSUM"))
            psb = ctx.enter_context(tc.tile_pool(name="psb", bufs=2, space="PSUM"))
            pst = ctx.enter_context(tc.tile_pool(name="pst", bufs=2, space="PSUM"))

            # ---- fixed tensors ----
            wqkv_sb = fixed.tile([128, 8, 3 * E], f8)
            for t in range(8):
                nc.sync.dma_start(out=wqkv_sb[:, t, :], in_=wqkv_r[:, t, :])
            proj_sb = fixed.tile([128, 8, E], f8)
            ffw2_sb = fixed.tile([128, 8, E], bf16)
            for t in range(8):
                nc.sync.dma_start(out=proj_sb[:, t, :], in_=proj_r[:, t, :])
                nc.sync.dma_start(out=ffw2_sb[:, t, :], in_=ffw2_r[:, t, :])
            bqkv_sb = fixed.tile([1, 3 * E], bf16)
            nc.sync.dma_start(out=bqkv_sb, in_=bqkv_d)
            bproj_sb = fixed.tile([1, E], bf16)
            nc.sync.dma_start(out=bproj_sb, in_=bproj_d)
            bff2_sb = fixed.tile([1, E], bf16)
            nc.sync.dma_start(out=bff2_sb, in_=bff2_d)
            if apply_gb:
                g_rep = fixed.tile([128, E], f32)
                nc.sync.dma_start(out=g_rep, in_=g_rep_d)
                b_rep = fixed.tile([128, E], f32)
                nc.sync.dma_start(out=b_rep, in_=b_rep_d)
            ones_row = fixed.tile([1, 128], bf16)
            nc.vector.memset(ones_row, 1.0)
            ident = fixed.tile([128, 128], bf16)
            make_identity(nc, ident)
            ident16 = fixed.tile([128, 128], bf16)
            make_identity(nc, ident16)
            nc.vector.tensor_scalar_mul(ident16, ident16, 16.0)
            eps_sb = fixed.tile([128, 1], f32)
            nc.vector.memset(eps_sb, EPS)
            eps256 = fixed.tile([128, 1], f32)
            nc.vector.memset(eps256, 256.0 * EPS)

            def layer_norm_rs(z, rs_out, mrs_out, scratch_bf, eps_t):
                """rsigma and -mu*rsigma of z [128, E]; stats on ScalarE,
                scalar TT arithmetic on GpSimd."""
                s1 = stats.tile([128, 1], f32, tag="s1")
                s2 = stats.tile([128, 1], f32, tag="s2")
                nc.scalar.activation(scratch_bf, z, ACT.Identity, accum_out=s1)
                nc.scalar.activation(scratch_bf, z, ACT.Square, accum_out=s2)
                mun = stats.tile([128, 1], f32, tag="mun")  # -mu
                nc.scalar.activation(mun, s1, ACT.Identity, scale=-inv_n)
                s2n = stats.tile([128, 1], f32, tag="s2n")  # s2/N
                nc.scalar.activation(s2n, s2, ACT.Identity, scale=inv_n)
                a = stats.tile([128, 1], f32, tag="a")  # mu^2
                nc.gpsimd.tensor_tensor(a, mun, mun, ALU.mult)
                var = stats.tile([128, 1], f32, tag="var")
                nc.gpsimd.tensor_tensor(var, s2n, a, ALU.subtract)
                lnv = stats.tile([128, 1], f32, tag="lnv")
                nc.scalar.activation(lnv, var, ACT.Ln, bias=eps_t)
                nc.scalar.activation(rs_out, lnv, ACT.Exp, scale=-0.5)
                nc.gpsimd.tensor_tensor(mrs_out, mun, rs_out, ALU.mult)

            prev = None
            for t in range(NT + 1):
                cur = None
                if t < NT:
                    s0 = t * 128
                    xf = work.tile([128, 8, 128], bf16, tag="xf")
                    nc.sync.dma_start(out=xf, in_=xT_r[:, :, s0 : s0 + 128])
                    xf8 = work.tile([128, 8, 128], f8, tag="xf8")
                    nc.sync.dma_start(out=xf8, in_=x8T_r[:, :, s0 : s0 + 128])
                    xb = work.tile([128, E], bf16, tag="xb")
                    nc.sync.dma_start(out=xb, in_=x_bf_d[s0 : s0 + 128, :])
                    cur = {"s0": s0, "xf": xf, "xb": xb}

                    # ---- qkv q,k groups first (PE) + evacs (Scalar) ----
                    qkv_sb = workq.tile([128, 3 * E], bf16, tag="qkv")

                    def qkv_group(j):
                        ps = psq.tile([128, 512], f32, tag="psq")
                        for e2 in range(4):
                            nc.tensor.matmul(
                                ps,
                                xf8[:, 2 * e2 : 2 * e2 + 2, :],
                                wqkv_sb[:, 2 * e2 : 2 * e2 + 2, j * 512 : (j + 1) * 512],
                                start=(e2 == 0),
                                stop=False,
                                perf_mode=mybir.MatmulPerfMode.DoubleRow,
                            )
                        nc.tensor.matmul(
                            ps,
                            ones_row,
                            bqkv_sb[:, j * 512 : (j + 1) * 512],
                            start=False,
                            stop=True,
                        )
                        nc.scalar.activation(
                            qkv_sb[:, j * 512 : (j + 1) * 512],
                            ps,
                            ACT.Identity,
                            scale=1.0 / 16.0,
                        )

                    for j in (0, 2, 3, 1):
                        qkv_group(j)

                    q3 = qkv_sb[:, 0:E].rearrange("p (h d) -> p h d", h=H)
                    k3 = qkv_sb[:, E : 2 * E].rearrange("p (g d) -> p g d", g=H)
                    v3 = qkv_sb[:, 2 * E : 3 * E].rearrange(
                        "p (d g) -> p d g", d=DV
                    )

                    # ---- QK bilinear per half (all on VectorE) ----
                    prod = work1.tile([128, 8, 16, 64], bf16, tag="prod")
                    scr = work1.tile([128, 8192], bf16, tag="scr")
                    scores = work.tile([128, H, H], f32, tag="scores")
                    p_sb = work.tile([128, H, H], bf16, tag="p_sb")
                    t1 = scr[:, 0:4096].rearrange("p (a g d) -> p a g d", a=8, g=16)
                    t2 = scr[:, 4096:6144].rearrange(
                        "p (a g d) -> p a g d", a=8, g=16
                    )
                    t3 = scr[:, 6144:7168].rearrange(
                        "p (a g d) -> p a g d", a=8, g=16
                    )

                    def qk_half(half):
                        h0 = half * 8
                        qb = (
                            q3[:, h0 : h0 + 8, :]
                            .unsqueeze(2)
                            .broadcast_to([128, 8, 16, 64])
                        )
                        kb = k3.unsqueeze(1).broadcast_to([128, 8, 16, 64])
                        nc.vector.tensor_tensor(prod, kb, qb, ALU.mult)
                        nc.vector.tensor_tensor(
                            t1, prod[:, :, :, 0:32], prod[:, :, :, 32:64], ALU.add
                        )
                        nc.vector.tensor_tensor(
                            t2, t1[:, :, :, 0:16], t1[:, :, :, 16:32], ALU.add
                        )
                        nc.vector.tensor_tensor(
                            t3, t2[:, :, :, 0:8], t2[:, :, :, 8:16], ALU.add
                        )
                        nc.vector.tensor_reduce(
                            scores[:, h0 : h0 + 8, :],
                            t3,
                            axis=mybir.AxisListType.X,
                            op=ALU.add,
                        )
                        # softmax exp for this half (Scalar)
                        nc.scalar.activation(
                            p_sb[:, h0 : h0 + 8, :],
                            scores[:, h0 : h0 + 8, :],
                            ACT.Exp,
                        )

                    qk_half(0)
                    qk_half(1)

                    # v groups (PE) + evacs after the exps in Scalar's queue
                    for j in (4, 5):
                        qkv_group(j)

                    # ---- softmax normalize + AV per half ----
                    attn_bf = work.tile([128, E], bf16, tag="attn_bf")
                    a3v = attn_bf.rearrange("p (h d) -> p h d", h=H)
                    phat = work1.tile([128, 2, 8, 16], bf16, tag="phat")
                    u1 = scr[:, 0:4096].rearrange("p (a d g) -> p a d g", a=8, d=64)
                    u2 = scr[:, 4096:6144].rearrange(
                        "p (a d g) -> p a d g", a=8, d=64
                    )
                    prod_flat = prod.rearrange("p a g d -> p (a g d)")
                    pa = prod_flat.rearrange("p (a d g) -> p a d g", a=8, d=DV)

                    def av_half(half):
                        h0 = half * 8
                        den = stats.tile([128, 8], f32, tag=f"den{half}")
                        nc.vector.tensor_reduce(
                            den,
                            p_sb[:, h0 : h0 + 8, :],
                            axis=mybir.AxisListType.X,
                            op=ALU.add,
                        )
                        rden = stats.tile([128, 8], bf16, tag=f"rden{half}")
                        with nc.allow_low_precision(reason="1/den in bf16"):
                            nc.vector.reciprocal(rden, den)
                        rb = rden.unsqueeze(2).broadcast_to([128, 8, 16])
                        nc.vector.tensor_tensor(
                            phat[:, half], p_sb[:, h0 : h0 + 8, :], rb, ALU.mult
                        )
                        pb = (
                            phat[:, half]
                            .unsqueeze(2)
                            .broadcast_to([128, 8, 64, 16])
                        )
                        vb = v3.unsqueeze(1).broadcast_to([128, 8, 64, 16])
                        nc.vector.tensor_tensor(pa, vb, pb, ALU.mult)
                        nc.vector.tensor_tensor(
                            u1, pa[:, :, :, 0:8], pa[:, :, :, 8:16], ALU.add
                        )
                        nc.vector.tensor_tensor(
                            u2, u1[:, :, :, 0:4], u1[:, :, :, 4:8], ALU.add
                        )
                        u3 = scr[:, 6144:7168].rearrange(
                            "p (a d g) -> p a d g", a=8, d=64
                        )
                        nc.vector.tensor_tensor(
                            u3, u2[:, :, :, 0:2], u2[:, :, :, 2:4], ALU.add
                        )
                        with nc.allow_low_precision(reason="2-term sum in bf16"):
                            nc.vector.tensor_reduce(
                                a3v[:, h0 : h0 + 8, :],
                                u3,
                                axis=mybir.AxisListType.X,
                                op=ALU.add,
                            )

                    av_half(0)
                    av_half(1)
                    cur["attn_bf"] = attn_bf

                if prev is not None:
                    p = prev
                    pxf, pxb = p["xf"], p["xb"]
                    # ---- attn transposes (packed psum bank) + evac ----
                    ptk = pst.tile([128, 8, 128], bf16, tag="pt")
                    for e in range(8):
                        nc.tensor.transpose(
                            ptk[:, e, :],
                            p["attn_bf"][:, e * 128 : (e + 1) * 128],
                            ident,
                        )
                    attn_fm = work.tile([128, 8, 128], f8, tag="attn_fm")
                    nc.scalar.copy(
                        attn_fm.rearrange("p a s -> p (a s)"),
                        ptk.rearrange("p a s -> p (a s)"),
                    )
                    # ---- proj + bias + residual in PSUM ----
                    ps2 = psb.tile([128, 1024], f32, tag="psb")
                    for j in range(2):
                        for e2 in range(4):
                            nc.tensor.matmul(
                                ps2[:, j * 512 : (j + 1) * 512],
                                attn_fm[:, 2 * e2 : 2 * e2 + 2, :],
                                proj_sb[:, 2 * e2 : 2 * e2 + 2, j * 512 : (j + 1) * 512],
                                start=(e2 == 0),
                                stop=False,
                                perf_mode=mybir.MatmulPerfMode.DoubleRow,
                            )
                        for c in range(4):
                            ec = 4 * j + c
                            nc.tensor.matmul(
                                ps2[:, ec * 128 : (ec + 1) * 128],
                                pxf[:, ec, :],
                                ident16,
                                start=False,
                                stop=False,
                                skip_group_check=True,
                            )
                        nc.tensor.matmul(
                            ps2[:, j * 512 : (j + 1) * 512],
                            ones_row,
                            bproj_sb[:, j * 512 : (j + 1) * 512],
                            start=False,
                            stop=True,
                        )
                    # ---- LN1 from PSUM ----
                    lnscr = work1.tile([128, E], bf16, tag="lnscr")
                    rs1 = stats.tile([128, 1], f32, tag="rs1")
                    mrs1 = stats.tile([128, 1], f32, tag="mrs1")
                    layer_norm_rs(ps2, rs1, mrs1, lnscr, eps256)
                    ln1_bf = work.tile([128, E], bf16, tag="ln1_bf")
                    nc.scalar.activation(
                        ln1_bf, ps2, ACT.Identity, bias=mrs1, scale=rs1
                    )
                    ptk2 = pst.tile([128, 8, 128], bf16, tag="pt")
                    for e in range(8):
                        nc.tensor.transpose(
                            ptk2[:, e, :], ln1_bf[:, e * 128 : (e + 1) * 128], ident
                        )
                    ln1_fm = work.tile([128, 8, 128], bf16, tag="ln1_fm")
                    nc.scalar.copy(
                        ln1_fm.rearrange("p a s -> p (a s)"),
                        ptk2.rearrange("p a s -> p (a s)"),
                    )
                    # ---- ff + gelu ----
                    ps3 = psb.tile([128, 1024], f32, tag="psb")
                    for j in range(2):
                        for e in range(8):
                            nc.tensor.matmul(
                                ps3[:, j * 512 : (j + 1) * 512],
                                ln1_fm[:, e, :],
                                ffw2_sb[:, e, j * 512 : (j + 1) * 512],
                                start=(e == 0),
                                stop=False,
                            )
                        nc.tensor.matmul(
                            ps3[:, j * 512 : (j + 1) * 512],
                            ones_row,
                            bff2_sb[:, j * 512 : (j + 1) * 512],
                            start=False,
                            stop=True,
                        )
                    gl = work.tile([128, E], bf16, tag="gl")
                    nc.scalar.activation(gl, ps3, ACT.Gelu)
                    # ---- second residual (VectorE; queued after attention) ----
                    z2 = work.tile([128, E], bf16, tag="z2")
                    nc.vector.tensor_tensor(z2, gl, pxb, ALU.add)
                    rs2 = stats.tile([128, 1], f32, tag="rs2")
                    mrs2 = stats.tile([128, 1], f32, tag="mrs2")
                    layer_norm_rs(z2, rs2, mrs2, lnscr, eps_sb)
                    out_t = work.tile([128, E], f32, tag="out_t")
                    if apply_gb:
                        zn = work1.tile([128, E], f32, tag="zn")
                        nc.scalar.activation(
                            zn, z2, ACT.Identity, bias=mrs2, scale=rs2
                        )
                        zn2 = work1.tile([128, E], f32, tag="zn2")
                        nc.gpsimd.tensor_tensor(zn2, zn, g_rep, ALU.mult)
                        nc.gpsimd.tensor_tensor(out_t, zn2, b_rep, ALU.add)
                    else:
                        nc.scalar.activation(
                            out_t, z2, ACT.Identity, bias=mrs2, scale=rs2
                        )
                    nc.sync.dma_start(
                        out=out_d[p["s0"] : p["s0"] + 128, :], in_=out_t
                    )

                prev = cur

    _split_excess_waits(nc)
    return nc


def _host_prep(inputs):
    x = np.asarray(inputs["x"], np.float32)
    qk_w = np.asarray(inputs["qk_w"], np.float32)
    qk_b = np.asarray(inputs["qk_b"], np.float32)
    v_w = np.asarray(inputs["v_w"], np.float32)
    v_b = np.asarray(inputs["v_b"], np.float32)
    proj_w = np.asarray(inputs["proj_w"], np.float32)
    proj_b = np.asarray(inputs["proj_b"], np.float32)
    ff_w = np.asarray(inputs["ff_w"], np.float32)
    ff_b = np.asarray(inputs["ff_b"], np.float32)
    ln_g = np.asarray(inputs["ln_g"], np.float32)
    ln_b = np.asarray(inputs["ln_b"], np.float32)

    apply_gb = not (np.all(ln_g == 1.0) and np.all(ln_b == 0.0))

    scale = 1.0 / np.sqrt(DQ).astype(np.float32)
    Wq = qk_w[:E] * scale
    bq = qk_b[:E] * scale
    Wk = qk_w[E:]
    bk = qk_b[E:]
    g_idx, d_idx = np.meshgrid(np.arange(H), np.arange(DV), indexing="ij")
    perm = np.empty(E, np.int64)
    perm[(d_idx * H + g_idx).ravel()] = (g_idx * DV + d_idx).ravel()
    Wv2 = v_w[perm]
    bv2 = v_b[perm]

    F8 = ml_dtypes.float8_e4m3fn
    # fp8 weights are stored x16 (better e4m3 resolution for |w|~0.03); the
    # 16x product scale is undone at PSUM evacuation (qkv) or folded into the
    # LN1 epsilon/stats math (proj, whose residual+bias are also x16).
    wqkvT = np.ascontiguousarray(
        (16.0 * np.concatenate([Wq, Wk, Wv2], 0).T).astype(F8)
    )  # [E, 3E]
    bqkv = (16.0 * np.concatenate([bq, bk, bv2]))[None, :].astype(BF)  # [1, 3E]
    projT = np.ascontiguousarray((16.0 * proj_w.T).astype(F8))  # [E, E]
    bproj = (16.0 * proj_b)[None, :].astype(BF)
    ffw2T = np.ascontiguousarray((ff_w * ln_g[None, :]).T.astype(BF))
    bff2 = (ff_b + ff_w @ ln_b)[None, :].astype(BF)

    shared = {
        "wqkvT": wqkvT,
        "bqkv": bqkv,
        "projT": projT,
        "bproj": bproj,
        "ffw2T": ffw2T,
        "bff2": bff2,
    }
    if apply_gb:
        shared["g_rep"] = np.ascontiguousarray(
            np.broadcast_to(ln_g[None, :], (128, E)), np.float32
        )
        shared["b_rep"] = np.ascontiguousarray(
            np.broadcast_to(ln_b[None, :], (128, E)), np.float32
        )
    in_maps = []
    for b in range(B):
        xb = np.ascontiguousarray(x[b])  # [S, E] f32
        xt = np.ascontiguousarray(xb.T)
        m = {
            "x_bf": xb.astype(BF),
            "xT": xt.astype(BF),
            "x8T": xt.astype(F8),
        }
        m.update(shared)
        in_maps.append(m)
    return in_maps, apply_gb


def kernel(**inputs) -> np.ndarray:
    from concourse.bass_utils import run_bass_kernel_spmd

    in_maps, apply_gb = _host_prep(inputs)
    key = ("nc", apply_gb)
    if key not in _CACHE:
        _CACHE[key] = _build_program(apply_gb)
        _CACHE["nc"] = _CACHE[key]  # convenience alias for external harnesses
    nc = _CACHE[key]

    res = run_bass_kernel_spmd(nc, in_maps, core_ids=list(range(B)))
    out = np.stack([res.results[b]["out"] for b in range(B)], 0)
    return out.astype(np.float32)


if __name__ == "__main__":
    rng = np.random.default_rng(0)
    ins = {
        "x": rng.standard_normal((B, S, E), np.float32),
        "qk_w": rng.standard_normal((2 * E, E), np.float32) * 0.03,
        "qk_b": rng.standard_normal((2 * E,), np.float32) * 0.03,
        "v_w": rng.standard_normal((E, E), np.float32) * 0.03,
        "v_b": rng.standard_normal((E,), np.float32) * 0.03,
        "proj_w": rng.standard_normal((E, E), np.float32) * 0.03,
        "proj_b": rng.standard_normal((E,), np.float32) * 0.03,
        "ff_w": rng.standard_normal((E, E), np.float32) * 0.03,
        "ff_b": rng.standard_normal((E,), np.float32) * 0.03,
        "ln_g": np.ones((E,), np.float32),
        "ln_b": np.zeros((E,), np.float32),
    }
    o = kernel(**ins)
    print("ran", o.shape, o.dtype)
